# revision 1
# baseline (speedup 1.0000x reference)
"""Trainium2 Bass kernel for nn_Attention_24704651887034.

Dense ViT-style attention block (B=16, N=1024, C=768, H=12 heads, 2D RoPE),
data-parallel over batch across 8 NeuronCores (2 batch items per core, no
collectives).

Per-core device kernel (all matmuls bf16, fp32 PSUM accumulation):
  A. xT: DMA-transpose x [N,C] -> feature-major xT [C,N] (bf16, xbar path)
  B. qkT = W_qk^T x^T computed feature-major [2C, N]; v computed token-major
     [N, C] with per-head 65-column layout [v_h | ones] for the fused
     softmax-denominator trick.
  C. 2D RoPE applied in feature-major layout: q' = q*cos + shuffle(q)*ssin
     where shuffle is the 16<->16 partition swap per 32-block (DVE
     stream_shuffle) and cos/ssin maps are host-precomputed per batch item.
  D. Per head: sT = k'q'^T (keys on partitions), exp via ScalarE (scale=1/8)
     into bf16 pT, outT[65,q] accumulated over key chunks with lhsT=[v|1]
     (row 64 = softmax denominators). Denominator rows are staged per batch
     via ACT copy + DMA row-gather, one batched DVE reciprocal, DMA
     row-broadcast, and an in-place bf16 multiply of the ao tiles.
  E. proj: out = attn_out @ W_proj, PSUM -> SBUF staging -> DRAM.
"""

import numpy as np

_B, _N, _C, _H = 16, 1024, 768, 12
_HD, _DR = 64, 32
_ROPE_BASE = 10000.0
_NCORES = 8
_BL = _B // _NCORES  # batch items per core

_NC6 = _C // 128      # 6 contraction chunks
_NF12 = 2 * _C // 128  # 12 feature chunks for q,k

_nc_cache = {}


def _split_excess_waits(nc, max_waits=1):
    """Walrus in this toolchain accepts at most one sync-wait command per
    instruction; Tile's tail drain (and occasionally the scheduler) emits
    more. Split the excess onto same-engine NOPs inserted just before."""
    from concourse import mybir

    for f in nc.m.functions:
        for blk in f.blocks:
            insts = blk.instructions
            i = 0
            while i < len(insts):
                ins = insts[i]
                si = getattr(ins, "sync_info", None)
                if si is not None and len(si.on_wait) > max_waits:
                    excess = si.on_wait[max_waits:]
                    ins.sync_info = mybir.SyncInfo(
                        on_wait=list(si.on_wait[:max_waits]),
                        on_update=list(si.on_update),
                    )
                    for j, w in enumerate(excess):
                        nop = mybir.InstNoOp(
                            name=f"{ins.name}-sw{j}", engine=ins.engine
                        )
                        nop.sync_info = mybir.SyncInfo(on_wait=[w], on_update=[])
                        insts.insert(i, nop)
                        i += 1
                i += 1
    return nc


def _build(has_bias):
    from contextlib import ExitStack

    import concourse.bass as bass
    import concourse.tile as tile
    from concourse import mybir

    BF = mybir.dt.bfloat16
    F32 = mybir.dt.float32
    Exp = mybir.ActivationFunctionType.Exp
    N, C, H = _N, _C, _H
    BL = _BL

    nc = bass.Bass("TRN2", target_bir_lowering=False, debug=False)
    x_d = nc.dram_tensor("x", [BL * N, C], BF, kind="ExternalInput").ap()
    wq_d = nc.dram_tensor("wq", [C, 3 * C], BF, kind="ExternalInput").ap()
    wp_d = nc.dram_tensor("wp", [C, C], BF, kind="ExternalInput").ap()
    cos_d = nc.dram_tensor("cosm", [BL * 128, N], BF, kind="ExternalInput").ap()
    sin_d = nc.dram_tensor("ssinm", [BL * 128, N], BF, kind="ExternalInput").ap()
    if has_bias:
        bqk_d = nc.dram_tensor("bqk", [1, 2 * C], BF, kind="ExternalInput").ap()
        bv_d = nc.dram_tensor("bv", [1, C], BF, kind="ExternalInput").ap()
        bp_d = nc.dram_tensor("bp", [1, C], BF, kind="ExternalInput").ap()
    out_d = nc.dram_tensor("out", [BL * N, C], F32, kind="ExternalOutput").ap()

    SH_MASK = [(i + 16) % 32 for i in range(32)]

    with tile.TileContext(nc) as tc, ExitStack() as ctx:
        const = ctx.enter_context(tc.tile_pool(name="const", bufs=1))
        xT_p = ctx.enter_context(tc.tile_pool(name="xT", bufs=7))
        map_p = ctx.enter_context(tc.tile_pool(name="maps", bufs=3))
        qkraw_p = ctx.enter_context(tc.tile_pool(name="qkraw", bufs=6))
        qkr_p = ctx.enter_context(tc.tile_pool(name="qkr", bufs=15))
        v_p = ctx.enter_context(tc.tile_pool(name="v", bufs=12))
        tmp_p = ctx.enter_context(tc.tile_pool(name="tmp", bufs=6))
        pT_p = ctx.enter_context(tc.tile_pool(name="pT", bufs=5))
        ao_p = ctx.enter_context(tc.tile_pool(name="ao", bufs=7))
        sum_p = ctx.enter_context(tc.tile_pool(name="sums", bufs=2))
        bc_p = ctx.enter_context(tc.tile_pool(name="bc", bufs=3))
        ost_p = ctx.enter_context(tc.tile_pool(name="ost", bufs=3))
        mm_ps = ctx.enter_context(tc.tile_pool(name="mmps", bufs=2, space="PSUM"))
        sc_ps = ctx.enter_context(tc.tile_pool(name="scps", bufs=2, space="PSUM"))
        o_ps = ctx.enter_context(tc.tile_pool(name="ops", bufs=2, space="PSUM"))

        # ---- resident constants: weights ----
        wq_t = []
        for c in range(_NC6):
            t = const.tile([128, 3 * C], BF, tag=f"wq{c}")
            nc.sync.dma_start(t[:], wq_d[c * 128:(c + 1) * 128, :])
            wq_t.append(t)
        wp_t = []
        for c in range(_NC6):
            t = const.tile([128, C], BF, tag=f"wp{c}")
            nc.sync.dma_start(t[:], wp_d[c * 128:(c + 1) * 128, :])
            wp_t.append(t)
        if has_bias:
            bqk_sb = const.tile([1, 2 * C], BF, tag="bqk")
            nc.sync.dma_start(bqk_sb[:], bqk_d[:])
            bv_sb = const.tile([1, C], BF, tag="bv")
            nc.sync.dma_start(bv_sb[:], bv_d[:])
            bp_sb = const.tile([1, C], BF, tag="bp")
            nc.sync.dma_start(bp_sb[:], bp_d[:])
            ones_r = const.tile([1, 512], BF, tag="ones")
            nc.gpsimd.memset(ones_r[:], 1.0)

        for b in range(BL):
            # ---- A: feature-major xT via xbar transpose DMA ----
            xT = []
            for c in range(_NC6):
                t = xT_p.tile([128, N], BF, tag="xT")
                nc.sync.dma_start(
                    t[:], x_d[b * N:(b + 1) * N, c * 128:(c + 1) * 128],
                    transpose=True,
                )
                xT.append(t)
            cosm = map_p.tile([128, N], BF, tag="cos")
            nc.sync.dma_start(cosm[:], cos_d[b * 128:(b + 1) * 128, :])
            ssin = map_p.tile([128, N], BF, tag="sin")
            nc.sync.dma_start(ssin[:], sin_d[b * 128:(b + 1) * 128, :])

            # ---- B1 + C: q,k feature-major + fused RoPE ----
            qk_r = []
            for f in range(_NF12):
                raw = qkraw_p.tile([128, N], BF, tag="qkraw")
                for t2 in range(2):
                    ps = mm_ps.tile([128, 512], F32, tag="mm", name="mmps1")
                    for c in range(_NC6):
                        nc.tensor.matmul(
                            ps[:],
                            wq_t[c][:, f * 128:(f + 1) * 128],
                            xT[c][:, t2 * 512:(t2 + 1) * 512],
                            start=(c == 0),
                            stop=(c == _NC6 - 1 and not has_bias),
                        )
                    if has_bias:
                        nc.tensor.matmul(
                            ps[:],
                            bqk_sb[:, f * 128:(f + 1) * 128],
                            ones_r[:],
                            start=False,
                            stop=True,
                        )
                    nc.scalar.copy(raw[:, t2 * 512:(t2 + 1) * 512], ps[:])
                r = tmp_p.tile([128, N], BF, tag="ttmp")
                nc.vector.stream_shuffle(r[:], raw[:], SH_MASK)
                tm = tmp_p.tile([128, N], BF, tag="ttmp")
                nc.vector.tensor_mul(tm[:], r[:], ssin[:])
                am = tmp_p.tile([128, N], BF, tag="ttmp")
                nc.gpsimd.tensor_mul(am[:], raw[:], cosm[:])
                ro = qkr_p.tile([128, N], BF, tag="qkr")
                nc.vector.tensor_add(ro[:], tm[:], am[:])
                qk_r.append(ro)

            # ---- B2: v token-major, per-head [v_h | ones] layout ----
            v_sb = []
            for t8 in range(8):
                vt = v_p.tile([128, H * 65], BF, tag="v")
                vt3 = vt.rearrange("p (h w) -> p h w", w=65)
                nc.gpsimd.memset(vt3[:, :, 64:65], 1.0)
                for f0, fw in ((0, 512), (512, 256)):
                    ps = mm_ps.tile([128, 512], F32, tag="mm", name="mmps2")
                    for c in range(_NC6):
                        nc.tensor.matmul(
                            ps[:, :fw],
                            xT[c][:, t8 * 128:(t8 + 1) * 128],
                            wq_t[c][:, 2 * C + f0:2 * C + f0 + fw],
                            start=(c == 0),
                            stop=(c == _NC6 - 1 and not has_bias),
                        )
                    if has_bias:
                        nc.tensor.matmul(
                            ps[:, :fw],
                            ones_r[:, t8 * 128 % 512:t8 * 128 % 512 + 128],
                            bv_sb[:, f0:f0 + fw],
                            start=False,
                            stop=True,
                        )
                    nh = fw // 64
                    nc.scalar.copy(
                        vt3[:, f0 // 64:f0 // 64 + nh, 0:64],
                        ps[:, :fw].rearrange("p (h w) -> p h w", w=64),
                    )
                v_sb.append(vt)

            # ---- D: attention, one head at a time, k-chunk streaming ----
            ao_t = [ao_p.tile([128, N], BF, tag="ao", name=f"ao{b}_{i}") for i in range(_NC6)]
            sums_st = sum_p.tile([H, N], F32, tag="sums", name=f"sums{b}")
            for h in range(H):
                jj, half = h // 2, (h % 2) * 64
                qh = qk_r[jj]
                kh = qk_r[6 + jj]
                op = [o_ps.tile([65, 512], F32, tag="o", name=f"op{b}_{h}_{i}") for i in range(2)]
                pT_tiles = [None] * 8
                for kc in range(9):
                    if kc < 8:
                        pT = pT_p.tile([128, N], BF, tag="pT", name=f"p{b}_{h}_{kc}")
                        s = sc_ps.tile([128, N], F32, tag="sc", name=f"s{b}_{h}_{kc}")
                        for qc in range(2):
                            nc.tensor.matmul(
                                s[:, qc * 512:(qc + 1) * 512],
                                kh[half:half + 64, kc * 128:(kc + 1) * 128],
                                qh[half:half + 64, qc * 512:(qc + 1) * 512],
                                start=True,
                                stop=True,
                            )
                        nc.scalar.activation(pT[:], s[:], Exp, scale=0.125)
                        pT_tiles[kc] = pT
                    if kc >= 1:
                        kd = kc - 1
                        for qc in range(2):
                            nc.tensor.matmul(
                                op[qc][:],
                                v_sb[kd][:, h * 65:(h + 1) * 65],
                                pT_tiles[kd][:, qc * 512:(qc + 1) * 512],
                                start=(kd == 0),
                                stop=(kd == 7),
                            )
                srow = sum_p.tile([1, N], F32, tag="srow", name=f"sr{b}_{h}")
                for qc in range(2):
                    nc.scalar.copy(
                        srow[:, qc * 512:(qc + 1) * 512], op[qc][64:65, :]
                    )
                nc.gpsimd.dma_start(sums_st[h:h + 1, :], srow[:])
                for qc in range(2):
                    nc.vector.tensor_copy(
                        ao_t[jj][half:half + 64, qc * 512:(qc + 1) * 512],
                        op[qc][0:64, :],
                    )

            # batched reciprocal of the 12 denominator rows, broadcast
            # each across 64 partitions (DMA), normalize ao tiles in place.
            recipf = sum_p.tile([H, N], F32, tag="recipf", name=f"rf{b}")
            nc.vector.reciprocal(recipf[:], sums_st[:])
            recipb = sum_p.tile([H, N], BF, tag="recipb", name=f"rb{b}")
            nc.vector.tensor_copy(recipb[:], recipf[:])
            for jj in range(_NC6):
                bch = bc_p.tile([128, N], BF, tag="bc", name=f"bc{b}_{jj}")
                for k in range(2):
                    nc.gpsimd.dma_start(
                        bch[k * 64:(k + 1) * 64, :],
                        recipb[2 * jj + k:2 * jj + k + 1, :]
                        .rearrange("p (u n) -> p u n", u=1)
                        .broadcast_to((1, 64, N)),
                    )
                nc.vector.tensor_mul(ao_t[jj][:], ao_t[jj][:], bch[:])

            # ---- E: output projection, PSUM -> SBUF staging -> DRAM ----
            for t8 in range(8):
                ot = ost_p.tile([128, C], F32, tag="ost")
                for nf in range(2):
                    ps = mm_ps.tile([128, 512], F32, tag="mm", name="mmps3")
                    for c in range(_NC6):
                        nc.tensor.matmul(
                            ps[:, :384],
                            ao_t[c][:, t8 * 128:(t8 + 1) * 128],
                            wp_t[c][:, nf * 384:(nf + 1) * 384],
                            start=(c == 0),
                            stop=(c == _NC6 - 1 and not has_bias),
                        )
                    if has_bias:
                        nc.tensor.matmul(
                            ps[:, :384],
                            ones_r[:, 0:128],
                            bp_sb[:, nf * 384:(nf + 1) * 384],
                            start=False,
                            stop=True,
                        )
                    nc.vector.tensor_copy(
                        ot[:, nf * 384:(nf + 1) * 384], ps[:, :384]
                    )
                nc.sync.dma_start(
                    out_d[b * N + t8 * 128:b * N + (t8 + 1) * 128, :], ot[:]
                )
    return _split_excess_waits(nc)


def _get_nc(has_bias):
    if has_bias not in _nc_cache:
        _nc_cache[has_bias] = _build(has_bias)
    return _nc_cache[has_bias]


def _prep_in_maps(x, W_qkv, b_qkv, W_proj, b_proj, pos_h, pos_w):
    import ml_dtypes

    bf16 = ml_dtypes.bfloat16
    has_bias = bool(np.any(b_qkv)) or bool(np.any(b_proj))

    inv = 1.0 / _ROPE_BASE ** (
        np.arange(0, _DR, 2, dtype=np.float32) / float(_DR)
    )  # [16]

    def rope_maps(pos):
        ang = pos.astype(np.float32)[..., None] * inv  # [B, N, 16]
        cos = np.repeat(np.cos(ang), 2, axis=-1)  # [B, N, 32]
        sin = np.repeat(np.sin(ang), 2, axis=-1)
        return cos.transpose(0, 2, 1), sin.transpose(0, 2, 1)  # [B, 32, N]

    ch, sh = rope_maps(np.asarray(pos_h))
    cw, sw = rope_maps(np.asarray(pos_w))
    cos64 = np.concatenate([ch, cw], axis=1)  # [B, 64, N]
    sin64 = np.concatenate([sh, sw], axis=1)
    sign = np.where((np.arange(64) % 32) < 16, -1.0, 1.0).astype(np.float32)
    ssin64 = sin64 * sign[None, :, None]
    cosm = np.tile(cos64, (1, 2, 1)).astype(bf16)  # [B, 128, N]
    ssinm = np.tile(ssin64, (1, 2, 1)).astype(bf16)

    xb = np.asarray(x).astype(bf16)
    wqb = np.ascontiguousarray(np.asarray(W_qkv).astype(bf16))
    wpb = np.ascontiguousarray(np.asarray(W_proj).astype(bf16))

    in_maps = []
    for i in range(_NCORES):
        lo, hi = i * _BL, (i + 1) * _BL
        m = {
            "x": np.ascontiguousarray(xb[lo:hi].reshape(_BL * _N, _C)),
            "wq": wqb,
            "wp": wpb,
            "cosm": np.ascontiguousarray(cosm[lo:hi].reshape(_BL * 128, _N)),
            "ssinm": np.ascontiguousarray(ssinm[lo:hi].reshape(_BL * 128, _N)),
        }
        if has_bias:
            bq = np.asarray(b_qkv).astype(bf16)
            m["bqk"] = np.ascontiguousarray(bq[:2 * _C].reshape(1, 2 * _C))
            m["bv"] = np.ascontiguousarray(bq[2 * _C:].reshape(1, _C))
            m["bp"] = np.ascontiguousarray(
                np.asarray(b_proj).astype(bf16).reshape(1, _C)
            )
        in_maps.append(m)
    return in_maps, has_bias


def _ensure_ntff_hook():
    """This image's antenv lacks axon_hooks; recreate it from the boot
    helper so run_bass_kernel_spmd(trace=True) can capture NTFF profiles."""
    import sys
    import types

    if "antenv.axon_hooks" in sys.modules:
        return
    try:
        from trn_agent_boot.trn_boot import _ntff_profile_via_ctypes

        hook = _ntff_profile_via_ctypes("/opt/axon/libaxon_pjrt.so")
    except Exception:
        hook = None
    mod = types.ModuleType("antenv.axon_hooks")
    mod._hook = hook
    mod.get_axon_ntff_profile_hook = lambda: mod._hook
    mod.set_axon_ntff_profile_hook = lambda h: setattr(mod, "_hook", h)
    sys.modules["antenv.axon_hooks"] = mod


def run(x, W_qkv, b_qkv, W_proj, b_proj, pos_h, pos_w, num_heads, **run_kwargs):
    """Build + execute on 8 NeuronCores; returns (output, BassKernelResults)."""
    from concourse.bass_utils import run_bass_kernel_spmd

    if run_kwargs.get("trace"):
        _ensure_ntff_hook()

    assert int(num_heads) == _H
    in_maps, has_bias = _prep_in_maps(
        x, W_qkv, b_qkv, W_proj, b_proj, pos_h, pos_w
    )
    nc = _get_nc(has_bias)
    res = run_bass_kernel_spmd(
        nc, in_maps, core_ids=list(range(_NCORES)), **run_kwargs
    )
    out = np.concatenate(
        [res.results[i]["out"].reshape(_BL, _N, _C) for i in range(_NCORES)],
        axis=0,
    ).astype(np.float32)
    return out, res


def kernel(x, W_qkv, b_qkv, W_proj, b_proj, pos_h, pos_w, num_heads):
    out, _ = run(x, W_qkv, b_qkv, W_proj, b_proj, pos_h, pos_w, num_heads)
    return out

